# revision 17
# baseline (speedup 1.0000x reference)
"""Trainium2 Bass kernel for MinibatchDiscrimination.

Reference computation (fp32):
    m = (x @ W.T + b).reshape(nb, 64, 16)            # nb=512
    d[i,j,B] = sum_c |m[i,B,c] - m[j,B,c]|
    o[i,B]   = sum_j exp(-d[i,j,B])
    out      = concat(x, o, axis=1)                   # (512, 1088)

Strategy: d is symmetric, so each unordered pair is computed once
globally.  The 512 rows form 16 blocks of 32; core c owns blocks c and
c+8.  Block r covers column-blocks r..r+8 (mod 16) if r<8 else
r..r+7 — a uniform circulant triangle.  With x row-rotated by 32c, core
c's rows are local rows 0..31 (window j in [0,288)) and 256..287
(window j in [256,512)) — identical program on every core, only data
differs.  Row sums come from the fused exp+accum; column partial sums
(excluding the self block, whose pairs both orderings compute) are
reduced on-chip by matmul against a 0/1 map and finished on the host.

All-fp16 pipeline (numpy-validated elementwise rel err ~2e-5: every
derived quantity is an exact function of the fp16-quantized m, so the
j==i diagonal is exact):

    mT[t] = W @ x^T + b  as 8 fp16 tiles [128 (B,c), 512 j]  (PE fp16)
Pairwise, |a-b| = a + b - 2*min(a,b) on 7 of 8 tiles (the DVE/gpsimd
TensorScalar ISA has min but no abs); tile 0 uses ACT's Abs activation
and is ordered last in each PSUM group so ACT's serial exp->abs chain
stays off the PE critical path:
    t!=0: minT = min(mT[t], m_i)   (DVE tensor_scalar 4x_2p ~194ns,
          gpsimd for t==3) -> fp16; psum += 2*csum_c minT (ind = +2)
    t==0: absT = Abs(m_i - mT[0])  (ACT) -> fp16; psum -= csum_c absT
With S = sum_c m over the 7 min-path tiles (psq = 2S via the same
indicator):  exp(-d) = exp(psum - S_i) * exp(-S_j):
    E    = Exp(psum + bias=-S_i)     (ACT, fp32)
    Escr = E*Q, o[:,p] = sum_j Escr  (DVE scalar_tensor_tensor accum)
    colsum += ones_map.T @ Escr      (PE, lag-2 emission)
Two local rows per PSUM tile (partitions 0-63 even i, 64-127 odd).

The container's walrus rejects instructions with >1 sync wait, so a
post-scheduling pass (_split_multi_waits) hoists extra waits onto
single-wait NoOps on the same engine queue.
"""

import sys
import numpy as np

if "/opt/trn_rl_repo" not in sys.path:
    sys.path.insert(0, "/opt/trn_rl_repo")

NB = 512          # batch rows
NIN = 1024        # n_in
NBF = 64          # n_B
NCD = 16          # n_C
FOUT = NBF * NCD  # 1024 projection features
NCORES = 8
RB = 32           # row block size; core c owns blocks c and c+8

T_ABS = 0         # feature tile on the ACT Abs path
T_POOL = 3        # feature tile on the gpsimd min path
T_ORDER = [1, 2, 4, 5, 6, 7, T_POOL, T_ABS]  # slow producers last

WTOP = 288        # j-window width for local rows 0..31
WBOT = 256        # j-window width for local rows 256..287 (at j=256)

_CACHE = {}


def _build_program():
    import concourse.bass as bass
    import concourse.tile as tile
    from concourse import mybir
    from contextlib import ExitStack

    f32 = mybir.dt.float32
    f16 = mybir.dt.float16
    Alu = mybir.AluOpType
    Act = mybir.ActivationFunctionType

    nc = bass.Bass()
    xT_d = nc.declare_dram_parameter("xT", [NIN, NB], f16, isOutput=False)
    wTt_d = nc.declare_dram_parameter("wTt", [8, 128, FOUT], f16, isOutput=False)
    b_d = nc.declare_dram_parameter("b", [128, 8], f32, isOutput=False)
    ind_d = nc.declare_dram_parameter("ind", [128, 8 * NBF], f16, isOutput=False)
    o_d = nc.declare_dram_parameter("o", [128, RB], f32, isOutput=True)
    csA_d = nc.declare_dram_parameter("csA", [128, WTOP - RB], f16, isOutput=True)
    csB_d = nc.declare_dram_parameter("csB", [128, WBOT - RB], f16, isOutput=True)

    with tile.TileContext(nc) as tc, ExitStack() as ctx:
        singles = ctx.enter_context(tc.tile_pool(name="singles", bufs=1))
        scratch = ctx.enter_context(tc.tile_pool(name="scratch", bufs=12))
        epool = ctx.enter_context(tc.tile_pool(name="epool", bufs=3))
        espool = ctx.enter_context(tc.tile_pool(name="espool", bufs=4))
        psM = ctx.enter_context(tc.tile_pool(name="psM", bufs=2, space="PSUM"))
        psP = ctx.enter_context(tc.tile_pool(name="psP", bufs=5, space="PSUM"))
        psQ = ctx.enter_context(tc.tile_pool(name="psQ", bufs=1, space="PSUM"))

        dma_sp = nc.default_dma_engine   # SP HWDGE queue
        dma_act = nc.scalar              # Activation HWDGE queue

        # ---- persistent loads -------------------------------------------
        wt_sb = [
            singles.tile([128, FOUT], f16, name=f"wt{t}", tag=f"wt{t}")
            for t in range(8)
        ]
        dma_act.dma_start(out=wt_sb[0], in_=wTt_d[0, :, :])
        xT_sb = []
        for kb in range(8):
            t_ = singles.tile([128, NB], f16, name=f"xT{kb}", tag=f"xT{kb}")
            dma_sp.dma_start(out=t_, in_=xT_d[128 * kb : 128 * (kb + 1), :])
            xT_sb.append(t_)
        for t in range(1, 8):
            dma_act.dma_start(out=wt_sb[t], in_=wTt_d[t, :, :])
        ind_sb = singles.tile([128, 8 * NBF], f16, name="ind", tag="ind")
        dma_sp.dma_start(out=ind_sb, in_=ind_d[:, :])
        b_sb = singles.tile([128, 8], f32, name="b_sb", tag="b_sb")
        dma_sp.dma_start(out=b_sb, in_=b_d[:, :])

        # ---- PE warmup: dependency-free matmuls fill the setup-DMA wait
        # so the tensor engine enters the m-build at full p-state.
        warm = singles.tile([128, NB], f16, name="warm", tag="warm")
        nc.vector.memset(warm, 0.0)
        wps = psM.tile([128, NB], f32, name="wps", tag="mps")
        for w in range(10):
            nc.tensor.matmul(
                wps, lhsT=warm[:, 0:128], rhs=warm,
                start=(w == 0), stop=(w == 9),
            )

        # ---- mT = W @ x^T + b as 8 fp16 tiles ---------------------------
        # mcol[t][:, 0:32] = local rows 0..31, [:, 32:64] = rows 256..287.
        mT16 = [None] * 8
        mcol = [None] * 8
        for t in range(8):
            ps = psM.tile([128, NB], f32, name="mps", tag="mps")
            for kb in range(8):
                nc.tensor.matmul(
                    ps, lhsT=wt_sb[t][:, 128 * kb : 128 * (kb + 1)],
                    rhs=xT_sb[kb], start=(kb == 0), stop=(kb == 7),
                )
            mt = singles.tile([128, NB], f16, name=f"mT{t}", tag=f"mT{t}")
            nc.scalar.activation(
                out=mt, in_=ps, func=Act.Identity,
                bias=b_sb[:, t : t + 1], scale=1.0,
            )
            mT16[t] = mt
            mc = singles.tile([128, 2 * RB], f32, name=f"mc{t}", tag=f"mc{t}")
            nc.vector.tensor_scalar_add(mc[:, 0:RB], mt[:, 0:RB], 0.0)
            nc.vector.tensor_scalar_add(mc[:, RB : 2 * RB], mt[:, 256 : 256 + RB], 0.0)
            mcol[t] = mc

        # ---- psq = 2*S over the 7 min-path tiles (fp16 ind = +2) --------
        psq = psQ.tile([NBF, NB], f32, name="psq", tag="psq")
        min_ts = [t for t in range(8) if t != T_ABS]
        for n, t in enumerate(min_ts):
            nc.tensor.matmul(
                psq, lhsT=ind_sb[0:128, NBF * t : NBF * (t + 1)], rhs=mT16[t],
                start=(n == 0), stop=(n == len(min_ts) - 1),
            )
        # negS2[64h+B, p] = -S[B, i(p,h)];  Q2[64h+B, j] = exp(-S[B, j])
        negS2 = singles.tile([128, RB], f32, name="negS2", tag="negS2")
        for blk, base in ((0, 0), (1, 256)):
            pairs = psq[:, base : base + RB].rearrange(
                "b (p two) -> b two p", two=2
            )
            for h in range(2):
                nc.scalar.activation(
                    out=negS2[NBF * h : NBF * (h + 1), 16 * blk : 16 * (blk + 1)],
                    in_=pairs[:, h, :], func=Act.Copy, bias=0.0, scale=-0.5,
                )
        Q2 = singles.tile([128, NB], f32, name="Q2", tag="Q2")
        nc.scalar.activation(out=Q2[0:NBF, :], in_=psq, func=Act.Exp,
                             bias=0.0, scale=-0.5)
        nc.scalar.activation(out=Q2[NBF:128, :], in_=psq, func=Act.Exp,
                             bias=0.0, scale=-0.5)

        oacc = singles.tile([128, RB], f32, name="oacc", tag="oacc")
        accA = singles.tile([128, WTOP - RB], f32, name="accA", tag="accA")
        accB = singles.tile([128, WBOT - RB], f32, name="accB", tag="accB")
        nc.gpsimd.memset(accA, 0.0)
        nc.gpsimd.memset(accB, 0.0)

        # ---- pairwise loop: p<16 -> rows 2p,2p+1 (window [0,288));
        #      p>=16 -> rows 256+2(p-16),+1 (window [256,512)).
        # stt lags 1 iteration and the colsum matmul 2 so PE/DVE never
        # stall on the exp->stt->colsum chain.
        E_q, Es_q = [], []  # (p, tile) pending queues

        def emit_stt(p, E, W, LO):
            Escr = espool.tile([128, W], f16, name="Escr", tag="Escr")
            nc.vector.scalar_tensor_tensor(
                out=Escr, in0=E, scalar=1.0, in1=Q2[:, LO : LO + W],
                op0=Alu.mult, op1=Alu.mult,
                accum_out=oacc[:, p : p + 1],
            )
            Es_q.append((p, Escr, W))

        def emit_cs(p, Escr, W):
            acc = accA if p < 16 else accB
            nc.gpsimd.tensor_tensor(
                out=acc[:, 0 : W - RB], in0=acc[:, 0 : W - RB],
                in1=Escr[:, RB:W], op=Alu.add,
            )

        # bottom block (p 16..31) first so its colsum drain overlaps the
        # top block's compute; only the top block's chain sits in the tail.
        P_ORDER = list(range(16, 32)) + list(range(16))
        for np_, p in enumerate(P_ORDER):
            top = p < 16
            LO, W = (0, WTOP) if top else (256, WBOT)
            psd = psP.tile([128, W], f32, name="psd", tag="psd")
            for half in range(2):
                icol = 2 * p + half if top else RB + 2 * (p - 16) + half
                for n_t, t in enumerate(T_ORDER):
                    dt_ = scratch.tile([128, W], f16, name="dt", tag="dt")
                    if t == T_ABS:
                        nc.scalar.activation(
                            out=dt_, in_=mT16[t][:, LO : LO + W], func=Act.Abs,
                            bias=mcol[t][:, icol : icol + 1], scale=-1.0,
                        )
                    else:
                        eng = nc.gpsimd if t == T_POOL else nc.vector
                        eng.tensor_scalar_min(
                            dt_, mT16[t][:, LO : LO + W],
                            mcol[t][:, icol : icol + 1],
                        )
                    nc.tensor.matmul(
                        psd[NBF * half : NBF * (half + 1), :],
                        lhsT=ind_sb[:, NBF * t : NBF * (t + 1)],
                        rhs=dt_, start=(n_t == 0), stop=(n_t == 7),
                    )
            E = epool.tile([128, W], f32, name="E", tag="E")
            nc.scalar.activation(
                out=E, in_=psd, func=Act.Exp,
                bias=negS2[:, p : p + 1], scale=1.0,
            )
            E_q.append((p, E, W, LO))
            if len(E_q) > 1:
                emit_stt(*E_q.pop(0))
            if len(Es_q) > 2:
                emit_cs(*Es_q.pop(0))
            if np_ == 18:
                # bottom-block chain has drained through the lag queues:
                # ship its outputs while the top block computes
                foldB = singles.tile([128, WBOT - RB], f16,
                                     name="foldB", tag="foldB")
                nc.gpsimd.tensor_scalar_add(foldB, accB, 0.0)
                dma_sp.dma_start(out=csB_d[:, :], in_=foldB)
                dma_sp.dma_start(out=o_d[:, 16:32], in_=oacc[:, 16:32])
        while E_q:
            emit_stt(*E_q.pop(0))
        while Es_q:
            emit_cs(*Es_q.pop(0))

        # downcast colsum accumulators to fp16; the host folds the two
        # i-parity halves (TensorTensor needs equal base partitions).
        foldA = singles.tile([128, WTOP - RB], f16, name="foldA", tag="foldA")
        nc.gpsimd.tensor_scalar_add(foldA, accA, 0.0)
        dma_sp.dma_start(out=csA_d[:, :], in_=foldA)
        dma_sp.dma_start(out=o_d[:, 0:16], in_=oacc[:, 0:16])

    _split_multi_waits(nc, mybir)
    return nc


def _split_multi_waits(nc, mybir):
    """This container's walrus rejects any instruction carrying more than
    one sync wait ("Too many sync wait commands").  Tile emits up to ~11.
    Legalize: hoist all but one wait onto single-wait NoOps inserted just
    before the instruction on the same engine queue (waits are sem-ge, so
    order is irrelevant; the queue blocks until all are satisfied)."""
    f = nc.m.functions[0]
    n_split = 0
    for blk in f.blocks:
        idx = 0
        while idx < len(blk.instructions):
            inst = blk.instructions[idx]
            si = inst.sync_info
            waits = list(si.on_wait) if si is not None and si.on_wait else []
            if len(waits) > 1:
                bysem = {}
                for w in waits:
                    k = w.id
                    if k not in bysem or (w.wait_value or 0) > (
                        bysem[k].wait_value or 0
                    ):
                        bysem[k] = w
                waits = list(bysem.values())
                for w in waits[:-1]:
                    nop = mybir.InstNoOp(
                        name=nc.get_next_instruction_name(), ins=[], outs=[]
                    )
                    nop.engine = inst.engine
                    nop.sync_info = mybir.SyncInfo(on_wait=[w], on_update=[])
                    blk.instructions.insert(idx, nop)
                    idx += 1
                    n_split += 1
                si.on_wait = [waits[-1]]
            idx += 1
    return n_split


def _get_program():
    if "nc" not in _CACHE:
        _CACHE["nc"] = _build_program()
    return _CACHE["nc"]


def _make_indicator():
    # ind[k, 64t + B] with B = 8t + k//16 (feature f=128t+k maps to B-col
    # f//16).  Min-path tiles get +2, the ACT abs tile -1; fp16 so the
    # pairwise matmuls run at full PE rate.
    ind = np.zeros((128, 8 * NBF), dtype=np.float16)
    for t in range(8):
        k = np.arange(128)
        val = -1.0 if t == T_ABS else 2.0
        ind[k, NBF * t + 8 * t + k // 16] = val
    return ind


def make_in_maps(x, W, b):
    x = np.ascontiguousarray(x, dtype=np.float32)
    W = np.ascontiguousarray(W, dtype=np.float32)
    b = np.ascontiguousarray(b, dtype=np.float32)
    ind = _make_indicator()
    # wTt[t, k, 128*kb + f] = W[128t+f, 128kb+k]
    wTt = np.ascontiguousarray(
        W.reshape(8, 128, 8, 128).transpose(0, 3, 2, 1).reshape(8, 128, FOUT)
    ).astype(np.float16)
    b_rs = np.ascontiguousarray(b.reshape(8, 128).T)
    in_maps = []
    for c in range(NCORES):
        xr = np.roll(x, -RB * c, axis=0)
        xT = np.ascontiguousarray(xr.T).astype(np.float16)
        in_maps.append({
            "xT": xT, "wTt": wTt, "b": b_rs, "ind": ind,
        })
    return in_maps


def kernel(x, W, b):
    from concourse.bass_utils import run_bass_kernel_spmd

    x = np.ascontiguousarray(x, dtype=np.float32)
    nc = _get_program()
    in_maps = make_in_maps(x, W, b)

    try:
        res = run_bass_kernel_spmd(nc, in_maps, list(range(NCORES)), trace=False)
    except Exception:
        # transient axon/NRT hiccups (e.g. device unrecoverable) resolve on
        # retry with a fresh dispatch
        res = run_bass_kernel_spmd(nc, in_maps, list(range(NCORES)), trace=False)
    _CACHE["last_results"] = res

    o_full = np.zeros((NB, NBF), dtype=np.float64)
    lrows = np.array(list(range(RB)) + list(range(256, 256 + RB)))
    for c in range(NCORES):
        g = (np.arange(NB) + RB * c) % NB   # local -> global row
        oc = res.results[c]["o"].astype(np.float64)      # [128, 32]
        # cols p<16: rows 2p,2p+1 ; p>=16: rows 256+2(p-16),+1
        for p in range(RB):
            for h in range(2):
                l = 2 * p + h if p < 16 else 256 + 2 * (p - 16) + h
                o_full[g[l]] += oc[NBF * h : NBF * (h + 1), p]
        csA = res.results[c]["csA"].astype(np.float64)   # [128, 256] cols 32..288
        csB = res.results[c]["csB"].astype(np.float64)   # [128, 224] cols 288..512
        o_full[g[(np.arange(WTOP - RB) + RB) % NB]] += (csA[0:NBF] + csA[NBF:128]).T
        o_full[g[(np.arange(WBOT - RB) + WTOP) % NB]] += (csB[0:NBF] + csB[NBF:128]).T
    return np.concatenate([x, o_full.astype(np.float32)], axis=1)
